# revision 51
# baseline (speedup 1.0000x reference)
"""Trainium2 Bass kernel for nn_Attention_34840774705279 (sparse/deformable attention).

Math (matches reference.py):
  v   = x @ v_w.T + v_b            -> per-head maps [B*NH, H, W, HD]
  off = x @ off_w.T + off_b        -> off_w is structurally zero, so offsets are
                                      CONSTANT per (head, point); for this problem
                                      they are (+-p or ~1e-16) => integer shifts.
  w   = softmax_p(x @ aw_w.T + aw_b)
  out[i,j] = sum_p w_p[i,j] * v[i+dy_p, j+dx_p]   (zero outside the map)
  y   = out @ proj_w.T + proj_b

Sharding (8 cores, uniform SPMD program):
  core d -> batch b = d//2, row-half r0 = 64*(d%2). Each core computes ALL 8
  heads for its 64 output rows (8192 tokens) using a 4-row halo of v rows
  (host zero-pads x rows outside the image); the host just concatenates.

Device algorithm (per core), bf16 datapath (tolerance is 2e-2; bf16 keeps
DVE in its 2x perf mode and matmuls at 1 cycle/row):
  A. v+logit projection, pixel-major: per image row r, xT chunks are the
     matmul stationary and [v_w.T | aw_w.T] streams, landing [col j, 288ch]
     in PSUM; evacuated (rotating over Scalar/Vector/Pool engines) into the
     d-major VL tile [j, 9 slots, 32 d, 72 rows] as bf16.
  B. softmax over the 4 points, batched across heads (exp on ScalarE,
     adds/reciprocal/normalize on VectorE), per row-half.
  C. sampling + weighting via weight-then-shift identity
        w .* (S_dx @ V_win) == S_dx @ ((S_-dx^T w) .* V_win):
     dx!=0 heads: the tiny bwd matmuls for ALL (head, point) of a half land
     in ONE PSUM tile, evacuated once; VectorE multiplies the V window
     (d-major, so the weight broadcast is on the middle axis and the 2x DVE
     mode stays on) and the 0/1 column-shift matmuls accumulate in PSUM.
     dx==0 heads (identity shift) skip PE entirely: multiply + add tree on
     VectorE writes the OUT tile directly.
  D. output projection: PE transposes OUT (bf16, both channel halves into
     one 2KB PSUM bank) back to channel-major, y^T = proj.T @ OUT^T; host
     transposes y^T back.
  Emission (software-pipelined, 16-row-quarter granularity throughout):
  A(0..5) | B0/E'0 quarter 0 | A(6..17) interleaved with a task queue of C0
  head-quarters and the later B/E' steps | D0 groups needing only written
  quarters interleave with the C0 tail and C1-q0 | D1 groups 0-3 (needing
  only C1-q0 rows) interleave with C1-q1 | D1 groups 4-7 tail. DMA count is
  minimized (xt in 5 chunked loads per half-channel, y flushed per 2 groups,
  consts in 2 blobs) since each DMA serializes on HWDGE/SP.SEQ.
"""

import os
import sys
import math

import numpy as np

sys.path.insert(0, "/opt/trn_rl_repo")

P = 128
H = W = 128
NH, NP, HD = 8, 4, 32
DIM = 256
N_TOK = H * W
ROWS_OUT = 64          # output rows per core
HALO = 4
ROWS_V = ROWS_OUT + 2 * HALO   # 72 v-row slots per core
TOK_V = ROWS_V * W             # 9216
N_CORES = 8
NCH = DIM + NH * NP    # 288

_cache = {}


def _build_terms(off_b):
    """Per (h, p): list of (dx, dy, alpha) corner terms from the constant offsets.

    General for any constant offset (bilinear corners); for this problem each
    (h, p) yields exactly one term with alpha ~= 1."""
    ob = np.asarray(off_b, np.float64).reshape(NH, NP, 2)
    terms = [[[] for _ in range(NP)] for _ in range(NH)]
    for h in range(NH):
        for p in range(NP):
            fx, fy = ob[h, p, 0], ob[h, p, 1]
            x0 = math.floor(fx)
            y0 = math.floor(fy)
            wx1 = fx - x0
            wy1 = fy - y0
            for dxc, wx in ((x0, 1.0 - wx1), (x0 + 1, wx1)):
                if abs(wx) < 1e-9:
                    continue
                for dyc, wy in ((y0, 1.0 - wy1), (y0 + 1, wy1)):
                    if abs(wy) < 1e-9:
                        continue
                    if abs(dxc) >= W or abs(dyc) > HALO:
                        continue  # fully out of range / beyond halo
                    terms[h][p].append((int(dxc), int(dyc), float(wx * wy)))
    return terms


def _build_smats(terms):
    """Dedupe (dx, alpha) -> [128,128] shift matrices; rewrite terms to
    (s_fwd, s_bwd, dy): out += S_dx @ (V_window * (alpha*S_-dx^T E))."""
    key_to_idx = {}
    mats = []

    def smat(dx, alpha):
        key = (dx, round(alpha, 9))
        if key not in key_to_idx:
            m = np.zeros((P, P), np.float32)
            for j_out in range(W):
                j_in = j_out + dx
                if 0 <= j_in < W:
                    m[j_in, j_out] = alpha
            key_to_idx[key] = len(mats)
            mats.append(m)
        return key_to_idx[key]

    terms2 = [[[] for _ in range(NP)] for _ in range(NH)]
    for h in range(NH):
        for p in range(NP):
            for dx, dy, alpha in terms[h][p]:
                terms2[h][p].append(
                    (smat(dx, 1.0), smat(-dx, alpha), dy))
    id_idx = key_to_idx.get((0, 1.0))
    return np.stack(mats, 0), terms2, id_idx


def _np_reference(x, v_w, v_b, aw_w, aw_b, off_w, off_b, proj_w, proj_b, Hh, Ww):
    """Pure-numpy fallback mirroring reference.py (used only if off_w != 0,
    which cannot happen with this problem's setup_inputs)."""
    B, N, C = x.shape
    v = (x @ v_w.T + v_b).reshape(B, N, NH, HD).transpose(0, 2, 1, 3)
    v = v.reshape(B * NH, Hh, Ww, HD)
    mh, mw = np.meshgrid(np.arange(Hh, dtype=x.dtype), np.arange(Ww, dtype=x.dtype),
                         indexing="ij")
    ref = np.stack([mw, mh], -1).reshape(1, N, 1, 2)
    off = (x @ off_w.T + off_b).reshape(B, N, NH, NP, 2).transpose(0, 2, 1, 3, 4)
    off = off.reshape(B * NH, N, NP, 2)
    grid = ref + off
    w = (x @ aw_w.T + aw_b).reshape(B, N, NH, NP).transpose(0, 2, 1, 3)
    w = w.reshape(B * NH, N, NP)
    w = np.exp(w - w.max(-1, keepdims=True))
    w = w / w.sum(-1, keepdims=True)
    G = B * NH
    vf = v.reshape(G, Hh * Ww, HD)
    gx, gy = grid[..., 0], grid[..., 1]
    x0 = np.floor(gx); y0 = np.floor(gy)
    wx1 = gx - x0; wx0 = 1.0 - wx1
    wy1 = gy - y0; wy0 = 1.0 - wy1
    x0i = x0.astype(np.int64); y0i = y0.astype(np.int64)

    def gather(xi, yi):
        valid = (xi >= 0) & (xi < Ww) & (yi >= 0) & (yi < Hh)
        idx = (np.clip(yi, 0, Hh - 1) * Ww + np.clip(xi, 0, Ww - 1))
        g = np.take_along_axis(vf, idx.reshape(G, -1, 1), axis=1)
        return g.reshape(*xi.shape, HD) * valid[..., None]

    samp = ((wy0 * wx0)[..., None] * gather(x0i, y0i)
            + (wy0 * wx1)[..., None] * gather(x0i + 1, y0i)
            + (wy1 * wx0)[..., None] * gather(x0i, y0i + 1)
            + (wy1 * wx1)[..., None] * gather(x0i + 1, y0i + 1))
    out = np.einsum("gnpd,gnp->gnd", samp, w)
    out = out.reshape(B, NH, N, HD).transpose(0, 2, 1, 3).reshape(B, N, C)
    return (out @ proj_w.T + proj_b).astype(np.float32)


def _classify_heads(terms, id_idx):
    """Heads whose every point is a single identity-column-shift term can be
    computed entirely on VectorE (no PE shift matmuls)."""
    dve_heads, mm_heads = [], []
    for h in range(NH):
        ok = id_idx is not None and all(
            len(terms[h][p]) == 1
            and terms[h][p][0][0] == id_idx and terms[h][p][0][1] == id_idx
            for p in range(NP))
        (dve_heads if ok else mm_heads).append(h)
    return dve_heads, mm_heads


def _build_program(terms, n_smats, has_bias, has_pbias):
    import concourse.bass as bass
    import concourse.mybir as mybir
    import concourse.tile as tile
    from concourse import bacc
    from concourse.ap import AP as RawAP

    dt = mybir.dt
    f32 = dt.float32
    bf16 = dt.bfloat16

    nc = bacc.Bacc("TRN2", target_bir_lowering=False, debug=False,
                   num_devices=N_CORES)

    _, _, id_idx_probe = None, None, None
    # id_idx is passed in via terms' construction; recompute head classes here
    # from the structural property instead (identity == s_fwd == s_bwd with a
    # dy-only shift). The caller passes id_idx through `terms` closure below.
    id_idx = _build_program._id_idx
    dve_heads, mm_heads = _classify_heads(terms, id_idx)

    def _affine_dy(h):
        """dy(p) = dy0 + p*dstep with one term per point -> batched mult."""
        if not all(len(terms[h][p]) == 1 for p in range(NP)):
            return None
        dys = [terms[h][p][0][2] for p in range(NP)]
        dstep = dys[1] - dys[0]
        if all(dys[p] == dys[0] + p * dstep for p in range(NP)):
            return (dys[0], dstep)
        return None

    affine = {h: _affine_dy(h) for h in range(NH)}
    # E' slot per (h, p, term_index) for mm heads
    ep_slot = {}
    n_slots = 0
    for h in mm_heads:
        for p in range(NP):
            for t in range(len(terms[h][p])):
                ep_slot[(h, p, t)] = n_slots
                n_slots += 1
    assert n_slots <= 32, "E' batch exceeds one PSUM pair; add chunking"

    NG = ROWS_V // 4           # 18 x-DMA groups of 4 rows
    NGO = ROWS_OUT // 4        # 16 groups for phase D

    # ---- DRAM I/O ----
    # consts blob columns: smats (n_smats*P) | pj (4*P) | id (P)
    CB = n_smats * P + 4 * P + P
    xt_d = nc.dram_tensor("xt_dev", [DIM, TOK_V], bf16, kind="ExternalInput")
    wb_d = nc.dram_tensor("wb_cat", [P, 2 * NCH], bf16, kind="ExternalInput")
    cb_d = nc.dram_tensor("consts", [P, CB], bf16, kind="ExternalInput")
    if has_pbias:
        pb_d = nc.dram_tensor("projb_t", [2, P], f32, kind="ExternalInput")
    if has_bias:
        ones_d = nc.dram_tensor("ones_dev", [1, TOK_V], bf16, kind="ExternalInput")
        bb_d = nc.dram_tensor("bb_cat", [1, NCH], bf16, kind="ExternalInput")
    y0_d = nc.dram_tensor("y0", [P, ROWS_OUT * W], f32, kind="ExternalOutput")
    y1_d = nc.dram_tensor("y1", [P, ROWS_OUT * W], f32, kind="ExternalOutput")
    y_outs = [y0_d, y1_d]

    with tile.TileContext(nc) as tc:
        with (
            tc.tile_pool(name="const", bufs=1) as cpool,
            tc.tile_pool(name="big", bufs=1) as bigpool,
            tc.tile_pool(name="stA", bufs=2) as stA,
            tc.tile_pool(name="wt", bufs=1) as wtpool,
            tc.tile_pool(name="stB", bufs=2) as stB,
        ):
            # ---- critical-path DMAs first: x^T chunks 1-2 and wb ----
            xt_sb = bigpool.tile([P, 2, TOK_V], bf16, tag="XT")
            XT_CHUNKS = [(0, 1), (1, 3), (3, 6), (6, 10), (10, NG)]
            for (g0, g1) in XT_CHUNKS[:1]:
                for kc in range(2):
                    nc.sync.dma_start(
                        xt_sb[:, kc, 512 * g0:512 * g1],
                        xt_d[P * kc:P * kc + P, 512 * g0:512 * g1])
            wbt = cpool.tile([P, 2 * NCH], bf16, tag="wbt")
            nc.gpsimd.dma_start(wbt[:], wb_d[:])
            wb_sb = wbt[:].rearrange("j (kc f) -> j kc f", kc=2)
            for (g0, g1) in XT_CHUNKS[1:2]:
                for kc in range(2):
                    nc.sync.dma_start(
                        xt_sb[:, kc, 512 * g0:512 * g1],
                        xt_d[P * kc:P * kc + P, 512 * g0:512 * g1])
            cb_sb = cpool.tile([P, CB], bf16, tag="cblob")
            nc.sync.dma_start(cb_sb[:], cb_d[:])
            o0 = 0
            s_sb = cb_sb[:, o0:o0 + n_smats * P].rearrange(
                "j (s f) -> j s f", s=n_smats)
            o0 += n_smats * P
            pj_sb = cb_sb[:, o0:o0 + 4 * P].rearrange(
                "j (kc m f) -> j kc m f", kc=2, m=2)
            o0 += 4 * P
            id_sb = cb_sb[:, o0:o0 + P]
            if has_pbias:
                pb_sb = cpool.tile([P, 2], f32, tag="projb")
                nc.sync.dma_start(pb_sb[:], pb_d.rearrange("m k -> k m"))
            if has_bias:
                bb_sb = cpool.tile([1, NCH], bf16, tag="bb")
                nc.sync.dma_start(bb_sb[:], bb_d[:])

            # ---- rest of x^T in large chunked DMAs ----
            for (g0, g1) in XT_CHUNKS[2:]:
                for kc in range(2):
                    nc.sync.dma_start(
                        xt_sb[:, kc, 512 * g0:512 * g1],
                        xt_d[P * kc:P * kc + P, 512 * g0:512 * g1])
            if has_bias:
                ones_sb = bigpool.tile([1, TOK_V], bf16, tag="ONES")
                nc.sync.dma_start(ones_sb[:], ones_d[:])

            # ---- persistent big tiles (all bf16, d-major) ----
            # vl: [j, slot(8 heads + logits), d, row]
            vl_sb = bigpool.tile([P, NH + 1, HD, ROWS_V], bf16, tag="V")
            v_sb = vl_sb[:, :NH]
            # outs[half]: [j, mc, hl, d, i]  (channel ch = mc*128+hl*32+d)
            outs = [bigpool.tile([P, 2, 4, HD, 32], bf16, tag="OUT", name="out0"),
                    bigpool.tile([P, 2, 4, HD, 32], bf16, tag="OUT2", name="out1")]
            es = [bigpool.tile([P, NH * NP, 32], bf16, tag="E", name="e0"),
                  bigpool.tile([P, NH * NP, 32], bf16, tag="E2", name="e1")]
            eps = [bigpool.tile([P, max(n_slots, 1), 32], bf16, tag="EP",
                                name="ep0"),
                   bigpool.tile([P, max(n_slots, 1), 32], bf16, tag="EP2",
                                name="ep1")]

            def phase_a(g, evac_eng):
                """x rows 4g..4g+4: v-proj + logits, pixel-major.
                PSUM is evacuated in row pairs to halve the fixed cost."""
                tok0 = g * 512
                for pr in range(2):
                    # rows padded to 512 so each matmul output stays inside
                    # one 2KB PSUM bank
                    a_ps = psA.tile([P, 2, 512], f32, tag="a_ps", bufs=3)
                    for rh in range(2):
                        rl = 2 * pr + rh
                        for kc in range(2):
                            nc.tensor.matmul(
                                a_ps[:, rh, :NCH],
                                xt_sb[:, kc, tok0 + P * rl:tok0 + P * rl + P],
                                wb_sb[:, kc, :], start=(kc == 0),
                                stop=(kc == 1 and not has_bias))
                        if has_bias:
                            nc.tensor.matmul(
                                a_ps[:, rh, :NCH],
                                ones_sb[:, tok0 + P * rl:tok0 + P * rl + P],
                                bb_sb[:], start=False, stop=True)
                    rr = 4 * g + 2 * pr
                    eng = evac_eng(rr)
                    dst = vl_sb[:, :, :, rr:rr + 2].rearrange(
                        "j s d r -> j r (s d)")
                    if eng is nc.scalar:
                        nc.scalar.copy(dst, a_ps[:, :, :NCH])
                    else:
                        eng.tensor_copy(dst, a_ps[:, :, :NCH])

            def phase_b(half, eng=None, quarter=None):
                """exp + softmax over points, all heads, rows of `half`
                (or one 16-row quarter of it)."""
                rr = 32 * half
                i0, ni = ((16 * quarter, 16) if quarter is not None
                          else (0, 32))
                e_sb = es[half]
                eng = eng or nc.vector
                nc.scalar.activation(
                    e_sb[:, :, i0:i0 + ni],
                    vl_sb[:, NH, :, HALO + rr + i0:HALO + rr + i0 + ni],
                    mybir.ActivationFunctionType.Exp)
                ev = e_sb[:].rearrange("j (h p) i -> j h p i",
                                       p=NP)[:, :, :, i0:i0 + ni]
                z_t = stB.tile([P, NH, 32], bf16, tag="z")
                zr_t = stB.tile([P, NH, 32], bf16, tag="zr")
                z2_t = stB.tile([P, NH, 2, 32], bf16, tag="z2")
                z = z_t[:, :, i0:i0 + ni]
                zr = zr_t[:, :, i0:i0 + ni]
                z2 = z2_t[:, :, :, i0:i0 + ni]
                with nc.allow_low_precision(reason="softmax denom in bf16; "
                                            "tolerance is 2e-2"):
                    eng.tensor_tensor(z2, ev[:, :, 0:2, :], ev[:, :, 2:4, :],
                                      op=mybir.AluOpType.add)
                    eng.tensor_tensor(z, z2[:, :, 0, :], z2[:, :, 1, :],
                                      op=mybir.AluOpType.add)
                    nc.vector.reciprocal(zr, z)
                    eng.tensor_tensor(
                        ev, ev,
                        zr.unsqueeze(2).broadcast_to([P, NH, NP, ni]),
                        op=mybir.AluOpType.mult)

            def phase_e(half, quarter=None):
                """Batched E' = alpha * S_-dx^T E for all mm-head terms.
                Borrows the oacc PSUM rotation (same bank pair)."""
                if n_slots == 0:
                    return
                i0, ni = ((16 * quarter, 16) if quarter is not None
                          else (0, 32))
                e_sb = es[half]
                ep_ps = psO.tile([P, 32, 32], f32, tag="oacc", bufs=1,
                                 name="ep_ps")
                for h in mm_heads:
                    for p in range(NP):
                        for t, (s_fwd, s_bwd, dy) in enumerate(terms[h][p]):
                            nc.tensor.matmul(
                                ep_ps[:, ep_slot[(h, p, t)], i0:i0 + ni],
                                s_sb[:, s_bwd, :],
                                e_sb[:, NP * h + p, i0:i0 + ni],
                                start=True, stop=True)
                nc.scalar.copy(eps[half][:, :n_slots, i0:i0 + ni],
                               ep_ps[:, :n_slots, i0:i0 + ni])

            def v4_view(h, slot00, dstep, ni=32):
                """Overlapping [j, p, d, i] view of the head's V window:
                slot index = slot00 + p*dstep + i (dy affine in p)."""
                base = vl_sb[:]
                off = base.offset + h * (HD * ROWS_V) + slot00
                dims = [list(base.ap[0]), [dstep, NP], [ROWS_V, HD], [1, ni]]
                return RawAP(base.tensor, off, dims)

            def phase_c(half, h, out_evac_eng, quarter=None):
                """Per-head sampling: weight-then-shift (mm) or pure-DVE.
                quarter: process a 16-row quarter (half0 early-start)."""
                rr = 32 * half
                i0, ni = ((16 * quarter, 16) if quarter is not None
                          else (0, 32))
                e_sb = es[half]
                mh, hl = h // 4, h % 4
                dst = outs[half][:, mh, hl, :, i0:i0 + ni]
                aff = affine[h]
                if h in dve_heads:
                    # out = sum_p E_p .* V_win(dy_p): one batched multiply on
                    # VectorE, add tree on Pool (SBUF-only ops legal there)
                    dy0, dstep = aff
                    if quarter is not None:
                        m4_t = wtpool.tile([P, NP, HD, 16], bf16, tag="dv4q",
                                           bufs=2, name="dv4q")
                    else:
                        m4_t = wtpool.tile([P, NP, HD, 32], bf16, tag="dv4",
                                           bufs=2, name="dv4")
                    m4 = m4_t[:]
                    with nc.allow_low_precision(reason="bf16 sampling"):
                        nc.vector.tensor_tensor(
                            m4, v4_view(h, rr + i0 + dy0 + HALO, dstep, ni),
                            e_sb[:, NP * h:NP * h + NP, i0:i0 + ni]
                            .unsqueeze(2).broadcast_to([P, NP, HD, ni]),
                            op=mybir.AluOpType.mult)
                    a2_t = wtpool.tile([P, 2, HD, 32], bf16, tag="dva",
                                       bufs=2, name="dva")
                    a2 = a2_t[:, :, :, :ni]
                    with nc.allow_low_precision(reason="bf16 acc"):
                        nc.gpsimd.tensor_tensor(a2, m4[:, 0:2], m4[:, 2:4],
                                                op=mybir.AluOpType.add)
                        nc.gpsimd.tensor_tensor(dst, a2[:, 0], a2[:, 1],
                                                op=mybir.AluOpType.add)
                    return
                # mm path
                n_terms = sum(len(terms[h][p]) for p in range(NP))
                if n_terms == 0:
                    nc.gpsimd.memset(dst, 0.0)
                    return
                if quarter is not None:
                    o_ps = psO.tile([P, HD, 16], f32, tag="oacc", bufs=1,
                                    name="o_ps_q")
                else:
                    o_ps = psO.tile([P, HD, 32], f32, tag="oacc", bufs=1)
                if aff is not None:
                    dy0, dstep = aff
                    base_slot = ep_slot[(h, 0, 0)]
                    if quarter is not None:
                        m4_t = wtpool.tile([P, NP, HD, 16], bf16, tag="wt4q",
                                           bufs=2, name="wt4q")
                    else:
                        m4_t = wtpool.tile([P, NP, HD, 32], bf16, tag="wt4",
                                           bufs=2, name="wt4")
                    m4 = m4_t[:]
                    with nc.allow_low_precision(reason="bf16 sampling"):
                        nc.vector.tensor_tensor(
                            m4, v4_view(h, rr + i0 + dy0 + HALO, dstep, ni),
                            eps[half][:, base_slot:base_slot + NP, i0:i0 + ni]
                            .unsqueeze(2).broadcast_to([P, NP, HD, ni]),
                            op=mybir.AluOpType.mult)
                    for p in range(NP):
                        s_fwd = terms[h][p][0][0]
                        if quarter is not None:
                            nc.tensor.matmul(
                                o_ps[:].rearrange("j d i -> j (d i)"),
                                s_sb[:, s_fwd, :],
                                m4[:, p].rearrange("j d i -> j (d i)"),
                                start=(p == 0), stop=(p == NP - 1))
                            continue
                        for ch in range(2):
                            nc.tensor.matmul(
                                o_ps[:, 16 * ch:16 * ch + 16, :]
                                .rearrange("j d i -> j (d i)"),
                                s_sb[:, s_fwd, :],
                                m4[:, p, 16 * ch:16 * ch + 16, :]
                                .rearrange("j d i -> j (d i)"),
                                start=(p == 0), stop=(p == NP - 1))
                else:
                    assert quarter is None
                    t_seen = 0
                    for p in range(NP):
                        for t, (s_fwd, s_bwd, dy) in enumerate(terms[h][p]):
                            slot0 = rr + dy + HALO
                            m_t = wtpool.tile([P, HD, 32], bf16,
                                              tag=f"wt{t_seen % 2}", bufs=2,
                                              name=f"mt{t_seen % 2}")
                            with nc.allow_low_precision(reason="bf16 sampling"):
                                nc.vector.tensor_tensor(
                                    m_t[:], v_sb[:, h, :, slot0:slot0 + 32],
                                    eps[half][:, ep_slot[(h, p, t)], :]
                                    .unsqueeze(1).broadcast_to([P, HD, 32]),
                                    op=mybir.AluOpType.mult)
                            for ch in range(2):
                                nc.tensor.matmul(
                                    o_ps[:, 16 * ch:16 * ch + 16, :]
                                    .rearrange("j d i -> j (d i)"),
                                    s_sb[:, s_fwd, :],
                                    m_t[:, 16 * ch:16 * ch + 16, :]
                                    .rearrange("j d i -> j (d i)"),
                                    start=(t_seen == 0),
                                    stop=(t_seen == n_terms - 1))
                            t_seen += 1
                eng = out_evac_eng
                if eng is nc.scalar:
                    nc.scalar.copy(dst, o_ps[:])
                else:
                    eng.tensor_copy(dst, o_ps[:])

            y_chunks = {}

            def phase_d(halfd, gl, ot_eng, y_eng, obufs=2, ybufs=2):
                """output projection for one 4-row group of half `halfd`.
                y rows collect into a chunk tile, DMA'd out per 4 groups."""
                g = halfd * (NGO // 2) + gl
                i0 = 4 * gl
                ot_ps = psD.tile([P, 2, 4, P], bf16, tag="ot", bufs=obufs)
                for kc in range(2):
                    for c in range(4):
                        nc.tensor.transpose(
                            ot_ps[:, kc, c, :],
                            outs[halfd][:, kc, :, :, i0 + c]
                            .rearrange("j hl d -> j (hl d)"),
                            id_sb[:])
                ot_sb = stA.tile([P, 2, 4, P], bf16, tag="ot_sb", bufs=2)
                if ot_eng is nc.scalar:
                    nc.scalar.copy(ot_sb[:], ot_ps[:])
                else:
                    ot_eng.tensor_copy(ot_sb[:], ot_ps[:])
                y_ps = psD.tile([P, 2, 512], f32, tag="yps", bufs=ybufs)
                for mc in range(2):
                    for kc in range(2):
                        nc.tensor.matmul(
                            y_ps[:, mc, :], pj_sb[:, kc, mc, :],
                            ot_sb[:, kc].rearrange("j c f -> j (c f)"),
                            start=(kc == 0), stop=(kc == 1))
                single = g >= 2 * NGO - 2
                if g % 2 == 0 or single:
                    y_chunks[0] = stA.tile([P, 2, 2, 512], f32, tag="ysb",
                                           name="ysb_ch", bufs=2)
                ysb_ch = y_chunks[0]
                ci = 0 if single else g % 2
                dst = ysb_ch[:, :, ci, :]
                if has_pbias:
                    for mc in range(2):
                        nc.scalar.activation(
                            dst[:, mc, :], y_ps[:, mc, :],
                            mybir.ActivationFunctionType.Identity,
                            bias=pb_sb[:, mc:mc + 1])
                elif y_eng is nc.scalar:
                    nc.scalar.copy(dst, y_ps[:])
                else:
                    y_eng.tensor_copy(dst, y_ps[:])
                if single:
                    for mc in range(2):
                        nc.sync.dma_start(
                            y_outs[mc][:, 512 * g:512 * (g + 1)],
                            ysb_ch[:, mc, 0, :])
                elif ci == 1:
                    for mc in range(2):
                        nc.sync.dma_start(
                            y_outs[mc][:, 1024 * (g // 2):1024 * (g // 2 + 1)],
                            ysb_ch[:, mc, :, :].rearrange("j c f -> j (c f)"))

            # preload the Exp activation table off the critical path
            warm = stB.tile([1, 2], bf16, tag="warm")
            nc.vector.memset(warm[:], 0.0)
            warm2 = stB.tile([1, 2], bf16, tag="warm2")
            nc.scalar.activation(warm2[:], warm[:],
                                 mybir.ActivationFunctionType.Exp)

            # ================= emission =================
            # A-evac engine: seg1 alternates Vector/Scalar; A-tail all Scalar
            seg1_rot = [nc.vector, nc.scalar]

            def a_evac_seg1(rr):
                if rr >= 24:
                    return nc.scalar
                return seg1_rot[(rr // 2) % 2]

            def a_evac_tail(rr):
                if rr >= 48:
                    return seg1_rot[(rr // 2) % 2]
                return nc.scalar

            cmA = tc.tile_pool(name="psA", bufs=1, space="PSUM")
            psA = cmA.__enter__()
            for g in range(6):
                phase_a(g, a_evac_seg1)
            phase_b(0, quarter=0)
            cmO = tc.tile_pool(name="psO", bufs=1, space="PSUM")
            psO = cmO.__enter__()
            phase_e(0, quarter=0)
            order0 = dve_heads + mm_heads
            tasks = []
            for i, h in enumerate(order0):
                if i == 7:
                    tasks.append(("bq1",))
                tasks.append(("c", h, 0))
            for i, h in enumerate(order0):
                if i == 6:
                    tasks.append(("b1",))
                tasks.append(("c", h, 1))

            def c0_step():
                if not tasks:
                    return
                t = tasks.pop(0)
                if t[0] == "bq1":
                    phase_b(0, quarter=1)
                    phase_e(0, quarter=1)
                elif t[0] == "b1":
                    phase_b(1, quarter=0)
                    phase_e(1, quarter=0)
                else:
                    phase_c(0, t[1], nc.scalar, quarter=t[2])

            for g in range(6, NG):
                phase_a(g, a_evac_tail)
                c0_step()
            cmO.__exit__(None, None, None)
            cmA.__exit__(None, None, None)

            cmO = tc.tile_pool(name="psO2", bufs=1, space="PSUM")
            psO = cmO.__enter__()
            cmD = tc.tile_pool(name="psD", bufs=1, space="PSUM")
            psD = cmD.__enter__()
            # D half-0 groups 0-3 need only the q0 rows: interleave them with
            # the remaining C0-q1 tail so y DMAs start as early as possible
            di = 0
            while tasks:
                c0_step()
                if di < 2:
                    phase_d(0, di, nc.scalar, nc.scalar)
                    di += 1
            # C1 in quarters: D0's tail overlaps C1-q0; D1 groups 0-3 (which
            # need only the C1-q0 rows) overlap C1-q1; only D1 groups 4-7
            # remain as pure tail.
            order1 = dve_heads + mm_heads
            for i in range(NH):
                if di < NGO // 2:
                    phase_d(0, di, nc.vector, nc.scalar)
                    di += 1
                phase_c(1, order1[i], nc.scalar, quarter=0)
            while di < NGO // 2:
                phase_d(0, di, nc.vector, nc.scalar)
                di += 1
            phase_b(1, quarter=1)
            phase_e(1, quarter=1)
            dj = 0
            for i in range(NH):
                phase_c(1, order1[i], nc.scalar, quarter=1)
                if i % 2 == 1 and dj < 4:
                    phase_d(1, dj, nc.vector, nc.scalar)
                    dj += 1
            while dj < NGO // 2:
                phase_d(1, dj, nc.vector, nc.scalar)
                dj += 1
            cmD.__exit__(None, None, None)
            cmO.__exit__(None, None, None)

    nc.compile()
    return nc


def kernel(x, v_w, v_b, aw_w, aw_b, off_w, off_b, proj_w, proj_b, H=128, W=128,
           **_unused):
    import ml_dtypes
    bf16 = ml_dtypes.bfloat16

    x = np.ascontiguousarray(np.asarray(x, np.float32))
    v_w = np.asarray(v_w, np.float32); v_b = np.asarray(v_b, np.float32)
    aw_w = np.asarray(aw_w, np.float32); aw_b = np.asarray(aw_b, np.float32)
    off_w = np.asarray(off_w, np.float32); off_b = np.asarray(off_b, np.float32)
    proj_w = np.asarray(proj_w, np.float32); proj_b = np.asarray(proj_b, np.float32)

    if np.any(off_w != 0.0) or int(H) != 128 or int(W) != 128:
        # data-dependent offsets or non-128 map: exact host fallback
        return _np_reference(x, v_w, v_b, aw_w, aw_b, off_w, off_b,
                             proj_w, proj_b, int(H), int(W))

    terms = _build_terms(off_b)
    s_mats, terms2, id_idx = _build_smats(terms)

    has_bias = bool(np.any(v_b) or np.any(aw_b))
    has_pbias = bool(np.any(proj_b))
    key = ("prog", s_mats.shape[0], has_bias, has_pbias, id_idx,
           tuple(tuple(tuple(tl) for tl in th) for th in terms2))
    if key not in _cache:
        _build_program._id_idx = id_idx
        _cache[key] = _build_program(terms2, s_mats.shape[0], has_bias,
                                     has_pbias)
    nc = _cache[key]

    B = x.shape[0]
    # ---- host prep, shared across cores ----
    wb_cat = np.empty((2, P, NCH), np.float32)
    for kc in range(2):
        wb_cat[kc, :, :256] = v_w[:, P * kc:P * (kc + 1)].T
        wb_cat[kc, :, 256:] = aw_w[:, P * kc:P * (kc + 1)].T
    pj_t = np.empty((2, 2, P, P), np.float32)
    for kc in range(2):
        for mc in range(2):
            pj_t[kc, mc] = proj_w[P * mc:P * (mc + 1), P * kc:P * (kc + 1)].T
    pb_t = proj_b.reshape(2, P)
    ident = np.eye(P, dtype=np.float32)
    blob = np.concatenate(
        [s_mats[s] for s in range(s_mats.shape[0])]
        + [pj_t[kc, mc] for kc in range(2) for mc in range(2)]
        + [ident], axis=1)
    shared = dict(
        consts=np.ascontiguousarray(blob.astype(bf16)),
        wb_cat=np.ascontiguousarray(
            np.concatenate([wb_cat[0], wb_cat[1]], axis=1).astype(bf16)))
    if has_pbias:
        shared["projb_t"] = np.ascontiguousarray(pb_t)
    if has_bias:
        bb_cat = np.concatenate([v_b, aw_b]).reshape(1, NCH)
        shared["bb_cat"] = np.ascontiguousarray(bb_cat.astype(bf16))

    xr = x.reshape(B, H, W, DIM)
    in_maps = []
    for d in range(N_CORES):
        b, half = d // 2, d % 2
        r0 = ROWS_OUT * half
        x_dev = np.zeros((ROWS_V, W, DIM), np.float32)
        lo, hi = max(0, r0 - HALO), min(H, r0 + ROWS_OUT + HALO)
        x_dev[lo - (r0 - HALO):hi - (r0 - HALO)] = xr[b, lo:hi]
        m = dict(shared)
        m["xt_dev"] = np.ascontiguousarray(
            x_dev.reshape(TOK_V, DIM).T.astype(bf16))
        if has_bias:
            ones = np.zeros((ROWS_V, W), np.float32)
            ones[lo - (r0 - HALO):hi - (r0 - HALO)] = 1.0
            m["ones_dev"] = ones.reshape(1, TOK_V).astype(bf16)
        in_maps.append(m)

    from concourse import bass_utils
    res = bass_utils.run_bass_kernel_spmd(
        nc, in_maps, core_ids=list(range(N_CORES)),
        trace=os.environ.get("KERNEL_TRACE", "0") == "1")
    kernel.last_results = res

    y = np.empty((B, N_TOK, DIM), np.float32)
    for d in range(N_CORES):
        b, half = d // 2, d % 2
        yd = np.concatenate([res.results[d]["y0"], res.results[d]["y1"]], 0)
        y[b, ROWS_OUT * W * half:ROWS_OUT * W * (half + 1), :] = yd.T
    return y


# revision 57
# speedup vs baseline: 1.0462x; 1.0462x over previous
"""Trainium2 Bass kernel for nn_Attention_34840774705279 (sparse/deformable attention).

Math (matches reference.py):
  v   = x @ v_w.T + v_b            -> per-head maps [B*NH, H, W, HD]
  off = x @ off_w.T + off_b        -> off_w is structurally zero, so offsets are
                                      CONSTANT per (head, point); for this problem
                                      they are (+-p or ~1e-16) => integer shifts.
  w   = softmax_p(x @ aw_w.T + aw_b)
  out[i,j] = sum_p w_p[i,j] * v[i+dy_p, j+dx_p]   (zero outside the map)
  y   = out @ proj_w.T + proj_b

Sharding (8 cores, uniform SPMD program):
  core d -> batch b = d//2, row-half r0 = 64*(d%2). Each core computes ALL 8
  heads for its 64 output rows (8192 tokens) using a 4-row halo of v rows
  (host zero-pads x rows outside the image); the host just concatenates.

Device algorithm (per core), bf16 datapath (tolerance is 2e-2; bf16 keeps
DVE in its 2x perf mode and matmuls at 1 cycle/row):
  A. v+logit projection, pixel-major: per image row r, xT chunks are the
     matmul stationary and [v_w.T | aw_w.T] streams, landing [col j, 288ch]
     in PSUM; evacuated (rotating over Scalar/Vector/Pool engines) into the
     d-major VL tile [j, 9 slots, 32 d, 72 rows] as bf16.
  B. softmax over the 4 points, batched across heads (exp on ScalarE,
     adds/reciprocal/normalize on VectorE), per row-half.
  C. sampling + weighting via weight-then-shift identity
        w .* (S_dx @ V_win) == S_dx @ ((S_-dx^T w) .* V_win):
     dx!=0 heads: the tiny bwd matmuls for ALL (head, point) of a half land
     in ONE PSUM tile, evacuated once; VectorE multiplies the V window
     (d-major, so the weight broadcast is on the middle axis and the 2x DVE
     mode stays on) and the 0/1 column-shift matmuls accumulate in PSUM.
     dx==0 heads (identity shift) skip PE entirely: multiply + add tree on
     VectorE writes the OUT tile directly.
  D. output projection: PE transposes OUT (bf16, both channel halves into
     one 2KB PSUM bank) back to channel-major, y^T = proj.T @ OUT^T; host
     transposes y^T back.
  Emission (software-pipelined, 16-row-quarter granularity throughout):
  A(0..5) | B0/E'0 quarter 0 | A(6..17) interleaved with a task queue of C0
  head-quarters and the later B/E' steps | D0 groups needing only written
  quarters interleave with the C0 tail and C1-q0 | D1 groups 0-3 (needing
  only C1-q0 rows) interleave with C1-q1 | D1 groups 4-7 tail. DMA count is
  minimized (xt in 5 chunked loads per half-channel, y flushed per 2 groups,
  consts in 2 blobs) since each DMA serializes on HWDGE/SP.SEQ.
"""

import os
import sys
import math

import numpy as np

sys.path.insert(0, "/opt/trn_rl_repo")

P = 128
H = W = 128
NH, NP, HD = 8, 4, 32
DIM = 256
N_TOK = H * W
ROWS_OUT = 64          # output rows per core
HALO = 4
ROWS_V = ROWS_OUT + 2 * HALO   # 72 v-row slots per core
TOK_V = ROWS_V * W             # 9216
N_CORES = 8
NCH = DIM + NH * NP    # 288

_cache = {}


def _build_terms(off_b):
    """Per (h, p): list of (dx, dy, alpha) corner terms from the constant offsets.

    General for any constant offset (bilinear corners); for this problem each
    (h, p) yields exactly one term with alpha ~= 1."""
    ob = np.asarray(off_b, np.float64).reshape(NH, NP, 2)
    terms = [[[] for _ in range(NP)] for _ in range(NH)]
    for h in range(NH):
        for p in range(NP):
            fx, fy = ob[h, p, 0], ob[h, p, 1]
            x0 = math.floor(fx)
            y0 = math.floor(fy)
            wx1 = fx - x0
            wy1 = fy - y0
            for dxc, wx in ((x0, 1.0 - wx1), (x0 + 1, wx1)):
                if abs(wx) < 1e-9:
                    continue
                for dyc, wy in ((y0, 1.0 - wy1), (y0 + 1, wy1)):
                    if abs(wy) < 1e-9:
                        continue
                    if abs(dxc) >= W or abs(dyc) > HALO:
                        continue  # fully out of range / beyond halo
                    terms[h][p].append((int(dxc), int(dyc), float(wx * wy)))
    return terms


def _build_smats(terms):
    """Dedupe (dx, alpha) -> [128,128] shift matrices; rewrite terms to
    (s_fwd, s_bwd, dy): out += S_dx @ (V_window * (alpha*S_-dx^T E))."""
    key_to_idx = {}
    mats = []

    def smat(dx, alpha):
        key = (dx, round(alpha, 9))
        if key not in key_to_idx:
            m = np.zeros((P, P), np.float32)
            for j_out in range(W):
                j_in = j_out + dx
                if 0 <= j_in < W:
                    m[j_in, j_out] = alpha
            key_to_idx[key] = len(mats)
            mats.append(m)
        return key_to_idx[key]

    terms2 = [[[] for _ in range(NP)] for _ in range(NH)]
    for h in range(NH):
        for p in range(NP):
            for dx, dy, alpha in terms[h][p]:
                terms2[h][p].append(
                    (smat(dx, 1.0), smat(-dx, alpha), dy))
    id_idx = key_to_idx.get((0, 1.0))
    return np.stack(mats, 0), terms2, id_idx


def _np_reference(x, v_w, v_b, aw_w, aw_b, off_w, off_b, proj_w, proj_b, Hh, Ww):
    """Pure-numpy fallback mirroring reference.py (used only if off_w != 0,
    which cannot happen with this problem's setup_inputs)."""
    B, N, C = x.shape
    v = (x @ v_w.T + v_b).reshape(B, N, NH, HD).transpose(0, 2, 1, 3)
    v = v.reshape(B * NH, Hh, Ww, HD)
    mh, mw = np.meshgrid(np.arange(Hh, dtype=x.dtype), np.arange(Ww, dtype=x.dtype),
                         indexing="ij")
    ref = np.stack([mw, mh], -1).reshape(1, N, 1, 2)
    off = (x @ off_w.T + off_b).reshape(B, N, NH, NP, 2).transpose(0, 2, 1, 3, 4)
    off = off.reshape(B * NH, N, NP, 2)
    grid = ref + off
    w = (x @ aw_w.T + aw_b).reshape(B, N, NH, NP).transpose(0, 2, 1, 3)
    w = w.reshape(B * NH, N, NP)
    w = np.exp(w - w.max(-1, keepdims=True))
    w = w / w.sum(-1, keepdims=True)
    G = B * NH
    vf = v.reshape(G, Hh * Ww, HD)
    gx, gy = grid[..., 0], grid[..., 1]
    x0 = np.floor(gx); y0 = np.floor(gy)
    wx1 = gx - x0; wx0 = 1.0 - wx1
    wy1 = gy - y0; wy0 = 1.0 - wy1
    x0i = x0.astype(np.int64); y0i = y0.astype(np.int64)

    def gather(xi, yi):
        valid = (xi >= 0) & (xi < Ww) & (yi >= 0) & (yi < Hh)
        idx = (np.clip(yi, 0, Hh - 1) * Ww + np.clip(xi, 0, Ww - 1))
        g = np.take_along_axis(vf, idx.reshape(G, -1, 1), axis=1)
        return g.reshape(*xi.shape, HD) * valid[..., None]

    samp = ((wy0 * wx0)[..., None] * gather(x0i, y0i)
            + (wy0 * wx1)[..., None] * gather(x0i + 1, y0i)
            + (wy1 * wx0)[..., None] * gather(x0i, y0i + 1)
            + (wy1 * wx1)[..., None] * gather(x0i + 1, y0i + 1))
    out = np.einsum("gnpd,gnp->gnd", samp, w)
    out = out.reshape(B, NH, N, HD).transpose(0, 2, 1, 3).reshape(B, N, C)
    return (out @ proj_w.T + proj_b).astype(np.float32)


def _classify_heads(terms, id_idx):
    """Heads whose every point is a single identity-column-shift term can be
    computed entirely on VectorE (no PE shift matmuls)."""
    dve_heads, mm_heads = [], []
    for h in range(NH):
        ok = id_idx is not None and all(
            len(terms[h][p]) == 1
            and terms[h][p][0][0] == id_idx and terms[h][p][0][1] == id_idx
            for p in range(NP))
        (dve_heads if ok else mm_heads).append(h)
    return dve_heads, mm_heads


def _build_program(terms, n_smats, has_bias, has_pbias):
    import concourse.bass as bass
    import concourse.mybir as mybir
    import concourse.tile as tile
    from concourse import bacc
    from concourse.ap import AP as RawAP

    dt = mybir.dt
    f32 = dt.float32
    bf16 = dt.bfloat16

    nc = bacc.Bacc("TRN2", target_bir_lowering=False, debug=False,
                   num_devices=N_CORES)

    _, _, id_idx_probe = None, None, None
    # id_idx is passed in via terms' construction; recompute head classes here
    # from the structural property instead (identity == s_fwd == s_bwd with a
    # dy-only shift). The caller passes id_idx through `terms` closure below.
    id_idx = _build_program._id_idx
    dve_heads, mm_heads = _classify_heads(terms, id_idx)

    def _affine_dy(h):
        """dy(p) = dy0 + p*dstep with one term per point -> batched mult."""
        if not all(len(terms[h][p]) == 1 for p in range(NP)):
            return None
        dys = [terms[h][p][0][2] for p in range(NP)]
        dstep = dys[1] - dys[0]
        if all(dys[p] == dys[0] + p * dstep for p in range(NP)):
            return (dys[0], dstep)
        return None

    affine = {h: _affine_dy(h) for h in range(NH)}
    # E' slot per (h, p, term_index) for mm heads
    ep_slot = {}
    n_slots = 0
    for h in mm_heads:
        for p in range(NP):
            for t in range(len(terms[h][p])):
                ep_slot[(h, p, t)] = n_slots
                n_slots += 1
    assert n_slots <= 32, "E' batch exceeds one PSUM pair; add chunking"

    NG = ROWS_V // 4           # 18 x-DMA groups of 4 rows
    NGO = ROWS_OUT // 4        # 16 groups for phase D

    # ---- DRAM I/O ----
    # consts split: smats load early (E' needs them ~8us); pj|id after x^T
    CB = n_smats * P
    CB2 = 4 * P + P
    xt_d = nc.dram_tensor("xt_dev", [DIM, TOK_V], bf16, kind="ExternalInput")
    wb_d = nc.dram_tensor("wb_cat", [P, 2 * NCH], bf16, kind="ExternalInput")
    cb_d = nc.dram_tensor("consts", [P, CB], bf16, kind="ExternalInput")
    cb2_d = nc.dram_tensor("consts2", [P, CB2], bf16, kind="ExternalInput")
    if has_pbias:
        pb_d = nc.dram_tensor("projb_t", [2, P], f32, kind="ExternalInput")
    if has_bias:
        ones_d = nc.dram_tensor("ones_dev", [1, TOK_V], bf16, kind="ExternalInput")
        bb_d = nc.dram_tensor("bb_cat", [1, NCH], bf16, kind="ExternalInput")
    y0_d = nc.dram_tensor("y0", [P, ROWS_OUT * W], f32, kind="ExternalOutput")
    y1_d = nc.dram_tensor("y1", [P, ROWS_OUT * W], f32, kind="ExternalOutput")
    y_outs = [y0_d, y1_d]

    with tile.TileContext(nc) as tc:
        with (
            tc.tile_pool(name="const", bufs=1) as cpool,
            tc.tile_pool(name="big", bufs=1) as bigpool,
            tc.tile_pool(name="stA", bufs=2) as stA,
            tc.tile_pool(name="wt", bufs=1) as wtpool,
            tc.tile_pool(name="stB", bufs=2) as stB,
        ):
            # ---- critical-path DMAs first: x^T chunks 1-2 and wb ----
            xt_sb = bigpool.tile([P, 2, TOK_V], bf16, tag="XT")
            XT_CHUNKS = [(0, 1), (1, 3), (3, 6), (6, 10), (10, NG)]
            for (g0, g1) in XT_CHUNKS[:1]:
                for kc in range(2):
                    nc.sync.dma_start(
                        xt_sb[:, kc, 512 * g0:512 * g1],
                        xt_d[P * kc:P * kc + P, 512 * g0:512 * g1])
            wbt = cpool.tile([P, 2 * NCH], bf16, tag="wbt")
            nc.gpsimd.dma_start(wbt[:], wb_d[:])
            wb_sb = wbt[:].rearrange("j (kc f) -> j kc f", kc=2)
            for (g0, g1) in XT_CHUNKS[1:4]:
                for kc in range(2):
                    nc.sync.dma_start(
                        xt_sb[:, kc, 512 * g0:512 * g1],
                        xt_d[P * kc:P * kc + P, 512 * g0:512 * g1])
            cb_sb = cpool.tile([P, CB], bf16, tag="cblob")
            nc.sync.dma_start(cb_sb[:], cb_d[:])
            s_sb = cb_sb[:].rearrange("j (s f) -> j s f", s=n_smats)
            cb2_sb = cpool.tile([P, CB2], bf16, tag="cblob2")
            pj_sb = cb2_sb[:, :4 * P].rearrange(
                "j (kc m f) -> j kc m f", kc=2, m=2)
            id_sb = cb2_sb[:, 4 * P:]
            if has_pbias:
                pb_sb = cpool.tile([P, 2], f32, tag="projb")
                nc.sync.dma_start(pb_sb[:], pb_d.rearrange("m k -> k m"))
            if has_bias:
                bb_sb = cpool.tile([1, NCH], bf16, tag="bb")
                nc.sync.dma_start(bb_sb[:], bb_d[:])

            # ---- rest of x^T in large chunked DMAs; pj|id after ----
            for (g0, g1) in XT_CHUNKS[4:]:
                for kc in range(2):
                    nc.sync.dma_start(
                        xt_sb[:, kc, 512 * g0:512 * g1],
                        xt_d[P * kc:P * kc + P, 512 * g0:512 * g1])
            nc.sync.dma_start(cb2_sb[:], cb2_d[:])
            if has_bias:
                ones_sb = bigpool.tile([1, TOK_V], bf16, tag="ONES")
                nc.sync.dma_start(ones_sb[:], ones_d[:])

            # ---- persistent big tiles (all bf16, d-major) ----
            # vl: [j, slot(8 heads + logits), d, row]
            vl_sb = bigpool.tile([P, NH + 1, HD, ROWS_V], bf16, tag="V")
            v_sb = vl_sb[:, :NH]
            # outs[half]: [j, mc, hl, d, i]  (channel ch = mc*128+hl*32+d)
            outs = [bigpool.tile([P, 2, 4, HD, 32], bf16, tag="OUT", name="out0"),
                    bigpool.tile([P, 2, 4, HD, 32], bf16, tag="OUT2", name="out1")]
            es = [bigpool.tile([P, NH * NP, 32], bf16, tag="E", name="e0"),
                  bigpool.tile([P, NH * NP, 32], bf16, tag="E2", name="e1")]
            eps = [bigpool.tile([P, max(n_slots, 1), 32], bf16, tag="EP",
                                name="ep0"),
                   bigpool.tile([P, max(n_slots, 1), 32], bf16, tag="EP2",
                                name="ep1")]

            def phase_a(g, evac_eng):
                """x rows 4g..4g+4: v-proj + logits, pixel-major.
                PSUM is evacuated in row pairs to halve the fixed cost."""
                tok0 = g * 512
                for pr in range(2):
                    # rows padded to 512 so each matmul output stays inside
                    # one 2KB PSUM bank
                    a_ps = psA.tile([P, 2, 512], f32, tag="a_ps", bufs=3)
                    for rh in range(2):
                        rl = 2 * pr + rh
                        for kc in range(2):
                            nc.tensor.matmul(
                                a_ps[:, rh, :NCH],
                                xt_sb[:, kc, tok0 + P * rl:tok0 + P * rl + P],
                                wb_sb[:, kc, :], start=(kc == 0),
                                stop=(kc == 1 and not has_bias))
                        if has_bias:
                            nc.tensor.matmul(
                                a_ps[:, rh, :NCH],
                                ones_sb[:, tok0 + P * rl:tok0 + P * rl + P],
                                bb_sb[:], start=False, stop=True)
                    rr = 4 * g + 2 * pr
                    eng = evac_eng(rr)
                    dst = vl_sb[:, :, :, rr:rr + 2].rearrange(
                        "j s d r -> j r (s d)")
                    if eng is nc.scalar:
                        nc.scalar.copy(dst, a_ps[:, :, :NCH])
                    else:
                        eng.tensor_copy(dst, a_ps[:, :, :NCH])

            def phase_b(half, eng=None, quarter=None):
                """exp + softmax over points, all heads, rows of `half`
                (or one 16-row quarter of it)."""
                rr = 32 * half
                i0, ni = ((16 * quarter, 16) if quarter is not None
                          else (0, 32))
                e_sb = es[half]
                eng = eng or nc.vector
                nc.scalar.activation(
                    e_sb[:, :, i0:i0 + ni],
                    vl_sb[:, NH, :, HALO + rr + i0:HALO + rr + i0 + ni],
                    mybir.ActivationFunctionType.Exp)
                ev = e_sb[:].rearrange("j (h p) i -> j h p i",
                                       p=NP)[:, :, :, i0:i0 + ni]
                z_t = stB.tile([P, NH, 32], bf16, tag="z")
                zr_t = stB.tile([P, NH, 32], bf16, tag="zr")
                z2_t = stB.tile([P, NH, 2, 32], bf16, tag="z2")
                z = z_t[:, :, i0:i0 + ni]
                zr = zr_t[:, :, i0:i0 + ni]
                z2 = z2_t[:, :, :, i0:i0 + ni]
                with nc.allow_low_precision(reason="softmax denom in bf16; "
                                            "tolerance is 2e-2"):
                    eng.tensor_tensor(z2, ev[:, :, 0:2, :], ev[:, :, 2:4, :],
                                      op=mybir.AluOpType.add)
                    eng.tensor_tensor(z, z2[:, :, 0, :], z2[:, :, 1, :],
                                      op=mybir.AluOpType.add)
                    nc.vector.reciprocal(zr, z)
                    eng.tensor_tensor(
                        ev, ev,
                        zr.unsqueeze(2).broadcast_to([P, NH, NP, ni]),
                        op=mybir.AluOpType.mult)

            def phase_e(half, quarter=None):
                """Batched E' = alpha * S_-dx^T E for all mm-head terms.
                Borrows the oacc PSUM rotation (same bank pair)."""
                if n_slots == 0:
                    return
                i0, ni = ((16 * quarter, 16) if quarter is not None
                          else (0, 32))
                e_sb = es[half]
                ep_ps = psO.tile([P, 32, 32], f32, tag="oacc", bufs=1,
                                 name="ep_ps")
                for h in mm_heads:
                    for p in range(NP):
                        for t, (s_fwd, s_bwd, dy) in enumerate(terms[h][p]):
                            nc.tensor.matmul(
                                ep_ps[:, ep_slot[(h, p, t)], i0:i0 + ni],
                                s_sb[:, s_bwd, :],
                                e_sb[:, NP * h + p, i0:i0 + ni],
                                start=True, stop=True)
                nc.scalar.copy(eps[half][:, :n_slots, i0:i0 + ni],
                               ep_ps[:, :n_slots, i0:i0 + ni])

            def v4_view(h, slot00, dstep, ni=32):
                """Overlapping [j, p, d, i] view of the head's V window:
                slot index = slot00 + p*dstep + i (dy affine in p)."""
                base = vl_sb[:]
                off = base.offset + h * (HD * ROWS_V) + slot00
                dims = [list(base.ap[0]), [dstep, NP], [ROWS_V, HD], [1, ni]]
                return RawAP(base.tensor, off, dims)

            def phase_c(half, h, out_evac_eng, quarter=None):
                """Per-head sampling: weight-then-shift (mm) or pure-DVE.
                quarter: process a 16-row quarter (half0 early-start)."""
                rr = 32 * half
                i0, ni = ((16 * quarter, 16) if quarter is not None
                          else (0, 32))
                e_sb = es[half]
                mh, hl = h // 4, h % 4
                dst = outs[half][:, mh, hl, :, i0:i0 + ni]
                aff = affine[h]
                if h in dve_heads:
                    # out = sum_p E_p .* V_win(dy_p): one batched multiply on
                    # VectorE, add tree on Pool (SBUF-only ops legal there)
                    dy0, dstep = aff
                    if quarter is not None:
                        m4_t = wtpool.tile([P, NP, HD, 16], bf16, tag="dv4q",
                                           bufs=2, name="dv4q")
                    else:
                        m4_t = wtpool.tile([P, NP, HD, 32], bf16, tag="dv4",
                                           bufs=2, name="dv4")
                    m4 = m4_t[:]
                    with nc.allow_low_precision(reason="bf16 sampling"):
                        nc.vector.tensor_tensor(
                            m4, v4_view(h, rr + i0 + dy0 + HALO, dstep, ni),
                            e_sb[:, NP * h:NP * h + NP, i0:i0 + ni]
                            .unsqueeze(2).broadcast_to([P, NP, HD, ni]),
                            op=mybir.AluOpType.mult)
                    a2_t = wtpool.tile([P, 2, HD, 32], bf16, tag="dva",
                                       bufs=2, name="dva")
                    a2 = a2_t[:, :, :, :ni]
                    with nc.allow_low_precision(reason="bf16 acc"):
                        nc.gpsimd.tensor_tensor(a2, m4[:, 0:2], m4[:, 2:4],
                                                op=mybir.AluOpType.add)
                        nc.gpsimd.tensor_tensor(dst, a2[:, 0], a2[:, 1],
                                                op=mybir.AluOpType.add)
                    return
                # mm path
                n_terms = sum(len(terms[h][p]) for p in range(NP))
                if n_terms == 0:
                    nc.gpsimd.memset(dst, 0.0)
                    return
                if quarter is not None:
                    o_ps = psO.tile([P, HD, 16], f32, tag="oacc", bufs=1,
                                    name="o_ps_q")
                else:
                    o_ps = psO.tile([P, HD, 32], f32, tag="oacc", bufs=1)
                if aff is not None:
                    dy0, dstep = aff
                    base_slot = ep_slot[(h, 0, 0)]
                    if quarter is not None:
                        m4_t = wtpool.tile([P, NP, HD, 16], bf16, tag="wt4q",
                                           bufs=2, name="wt4q")
                    else:
                        m4_t = wtpool.tile([P, NP, HD, 32], bf16, tag="wt4",
                                           bufs=2, name="wt4")
                    m4 = m4_t[:]
                    with nc.allow_low_precision(reason="bf16 sampling"):
                        nc.vector.tensor_tensor(
                            m4, v4_view(h, rr + i0 + dy0 + HALO, dstep, ni),
                            eps[half][:, base_slot:base_slot + NP, i0:i0 + ni]
                            .unsqueeze(2).broadcast_to([P, NP, HD, ni]),
                            op=mybir.AluOpType.mult)
                    for p in range(NP):
                        s_fwd = terms[h][p][0][0]
                        if quarter is not None:
                            nc.tensor.matmul(
                                o_ps[:].rearrange("j d i -> j (d i)"),
                                s_sb[:, s_fwd, :],
                                m4[:, p].rearrange("j d i -> j (d i)"),
                                start=(p == 0), stop=(p == NP - 1))
                            continue
                        for ch in range(2):
                            nc.tensor.matmul(
                                o_ps[:, 16 * ch:16 * ch + 16, :]
                                .rearrange("j d i -> j (d i)"),
                                s_sb[:, s_fwd, :],
                                m4[:, p, 16 * ch:16 * ch + 16, :]
                                .rearrange("j d i -> j (d i)"),
                                start=(p == 0), stop=(p == NP - 1))
                else:
                    assert quarter is None
                    t_seen = 0
                    for p in range(NP):
                        for t, (s_fwd, s_bwd, dy) in enumerate(terms[h][p]):
                            slot0 = rr + dy + HALO
                            m_t = wtpool.tile([P, HD, 32], bf16,
                                              tag=f"wt{t_seen % 2}", bufs=2,
                                              name=f"mt{t_seen % 2}")
                            with nc.allow_low_precision(reason="bf16 sampling"):
                                nc.vector.tensor_tensor(
                                    m_t[:], v_sb[:, h, :, slot0:slot0 + 32],
                                    eps[half][:, ep_slot[(h, p, t)], :]
                                    .unsqueeze(1).broadcast_to([P, HD, 32]),
                                    op=mybir.AluOpType.mult)
                            for ch in range(2):
                                nc.tensor.matmul(
                                    o_ps[:, 16 * ch:16 * ch + 16, :]
                                    .rearrange("j d i -> j (d i)"),
                                    s_sb[:, s_fwd, :],
                                    m_t[:, 16 * ch:16 * ch + 16, :]
                                    .rearrange("j d i -> j (d i)"),
                                    start=(t_seen == 0),
                                    stop=(t_seen == n_terms - 1))
                            t_seen += 1
                eng = out_evac_eng
                if eng is nc.scalar:
                    nc.scalar.copy(dst, o_ps[:])
                else:
                    eng.tensor_copy(dst, o_ps[:])

            y_chunks = {}

            def phase_d(halfd, gl, ot_eng, y_eng, obufs=2, ybufs=2):
                """output projection for one 4-row group of half `halfd`.
                y rows collect into a chunk tile, DMA'd out per 4 groups."""
                g = halfd * (NGO // 2) + gl
                i0 = 4 * gl
                ot_ps = psD.tile([P, 2, 4, P], bf16, tag="ot", bufs=obufs)
                for kc in range(2):
                    for c in range(4):
                        nc.tensor.transpose(
                            ot_ps[:, kc, c, :],
                            outs[halfd][:, kc, :, :, i0 + c]
                            .rearrange("j hl d -> j (hl d)"),
                            id_sb[:])
                ot_sb = stA.tile([P, 2, 4, P], bf16, tag="ot_sb", bufs=2)
                if ot_eng is nc.scalar:
                    nc.scalar.copy(ot_sb[:], ot_ps[:])
                else:
                    ot_eng.tensor_copy(ot_sb[:], ot_ps[:])
                y_ps = psD.tile([P, 2, 512], f32, tag="yps", bufs=ybufs)
                for mc in range(2):
                    for kc in range(2):
                        nc.tensor.matmul(
                            y_ps[:, mc, :], pj_sb[:, kc, mc, :],
                            ot_sb[:, kc].rearrange("j c f -> j (c f)"),
                            start=(kc == 0), stop=(kc == 1))
                single = g >= 2 * NGO - 2
                if g % 2 == 0 or single:
                    y_chunks[0] = stA.tile([P, 2, 2, 512], f32, tag="ysb",
                                           name="ysb_ch", bufs=2)
                ysb_ch = y_chunks[0]
                ci = 0 if single else g % 2
                dst = ysb_ch[:, :, ci, :]
                if has_pbias:
                    for mc in range(2):
                        nc.scalar.activation(
                            dst[:, mc, :], y_ps[:, mc, :],
                            mybir.ActivationFunctionType.Identity,
                            bias=pb_sb[:, mc:mc + 1])
                elif y_eng is nc.scalar:
                    nc.scalar.copy(dst, y_ps[:])
                else:
                    y_eng.tensor_copy(dst, y_ps[:])
                if single:
                    for mc in range(2):
                        nc.sync.dma_start(
                            y_outs[mc][:, 512 * g:512 * (g + 1)],
                            ysb_ch[:, mc, 0, :])
                elif ci == 1:
                    for mc in range(2):
                        nc.sync.dma_start(
                            y_outs[mc][:, 1024 * (g // 2):1024 * (g // 2 + 1)],
                            ysb_ch[:, mc, :, :].rearrange("j c f -> j (c f)"))

            # preload the Exp activation table off the critical path
            warm = stB.tile([1, 2], bf16, tag="warm")
            nc.vector.memset(warm[:], 0.0)
            warm2 = stB.tile([1, 2], bf16, tag="warm2")
            nc.scalar.activation(warm2[:], warm[:],
                                 mybir.ActivationFunctionType.Exp)

            # ================= emission =================
            # A-evac engine: seg1 alternates Vector/Scalar; A-tail all Scalar
            seg1_rot = [nc.vector, nc.scalar]

            def a_evac_seg1(rr):
                if rr >= 24:
                    return nc.scalar
                return seg1_rot[(rr // 2) % 2]

            def a_evac_tail(rr):
                if rr >= 52:
                    return seg1_rot[(rr // 2) % 2]
                return nc.scalar

            cmA = tc.tile_pool(name="psA", bufs=1, space="PSUM")
            psA = cmA.__enter__()
            for g in range(6):
                phase_a(g, a_evac_seg1)
            phase_b(0, quarter=0)
            cmO = tc.tile_pool(name="psO", bufs=1, space="PSUM")
            psO = cmO.__enter__()
            phase_e(0, quarter=0)
            order0 = dve_heads + mm_heads
            tasks = []
            for i, h in enumerate(order0):
                if i == 7:
                    tasks.append(("bq1",))
                tasks.append(("c", h, 0))
            for i, h in enumerate(order0):
                if i == 6:
                    tasks.append(("b1",))
                tasks.append(("c", h, 1))

            def c0_step():
                if not tasks:
                    return
                t = tasks.pop(0)
                if t[0] == "bq1":
                    phase_b(0, quarter=1)
                    phase_e(0, quarter=1)
                elif t[0] == "b1":
                    phase_b(1, quarter=0)
                    phase_e(1, quarter=0)
                else:
                    phase_c(0, t[1], nc.scalar, quarter=t[2])

            for g in range(6, NG):
                phase_a(g, a_evac_tail)
                c0_step()
            cmO.__exit__(None, None, None)
            cmA.__exit__(None, None, None)

            cmO = tc.tile_pool(name="psO2", bufs=1, space="PSUM")
            psO = cmO.__enter__()
            cmD = tc.tile_pool(name="psD", bufs=1, space="PSUM")
            psD = cmD.__enter__()
            # D half-0 groups 0-3 need only the q0 rows: interleave them with
            # the remaining C0-q1 tail so y DMAs start as early as possible
            di = 0
            while tasks:
                c0_step()
                if di < 0:
                    phase_d(0, di, nc.scalar, nc.scalar)
                    di += 1
            # C1 in quarters: D0's tail overlaps C1-q0; D1 groups 0-3 (which
            # need only the C1-q0 rows) overlap C1-q1; only D1 groups 4-7
            # remain as pure tail.
            order1 = dve_heads + mm_heads
            for i in range(NH):
                if di < NGO // 2:
                    phase_d(0, di, nc.vector, nc.scalar)
                    di += 1
                phase_c(1, order1[i], nc.scalar, quarter=0)
            while di < NGO // 2:
                phase_d(0, di, nc.vector, nc.scalar)
                di += 1
            phase_b(1, quarter=1)
            phase_e(1, quarter=1)
            dj = 0
            for i in range(NH):
                phase_c(1, order1[i], nc.scalar, quarter=1)
                if i % 2 == 1 and dj < 4:
                    phase_d(1, dj, nc.vector, nc.scalar)
                    dj += 1
            while dj < NGO // 2:
                phase_d(1, dj, nc.vector, nc.scalar)
                dj += 1
            cmD.__exit__(None, None, None)
            cmO.__exit__(None, None, None)

    nc.compile()
    return nc


def kernel(x, v_w, v_b, aw_w, aw_b, off_w, off_b, proj_w, proj_b, H=128, W=128,
           **_unused):
    import ml_dtypes
    bf16 = ml_dtypes.bfloat16

    x = np.ascontiguousarray(np.asarray(x, np.float32))
    v_w = np.asarray(v_w, np.float32); v_b = np.asarray(v_b, np.float32)
    aw_w = np.asarray(aw_w, np.float32); aw_b = np.asarray(aw_b, np.float32)
    off_w = np.asarray(off_w, np.float32); off_b = np.asarray(off_b, np.float32)
    proj_w = np.asarray(proj_w, np.float32); proj_b = np.asarray(proj_b, np.float32)

    if np.any(off_w != 0.0) or int(H) != 128 or int(W) != 128:
        # data-dependent offsets or non-128 map: exact host fallback
        return _np_reference(x, v_w, v_b, aw_w, aw_b, off_w, off_b,
                             proj_w, proj_b, int(H), int(W))

    terms = _build_terms(off_b)
    s_mats, terms2, id_idx = _build_smats(terms)

    has_bias = bool(np.any(v_b) or np.any(aw_b))
    has_pbias = bool(np.any(proj_b))
    key = ("prog", s_mats.shape[0], has_bias, has_pbias, id_idx,
           tuple(tuple(tuple(tl) for tl in th) for th in terms2))
    if key not in _cache:
        _build_program._id_idx = id_idx
        _cache[key] = _build_program(terms2, s_mats.shape[0], has_bias,
                                     has_pbias)
    nc = _cache[key]

    B = x.shape[0]
    # ---- host prep, shared across cores ----
    wb_cat = np.empty((2, P, NCH), np.float32)
    for kc in range(2):
        wb_cat[kc, :, :256] = v_w[:, P * kc:P * (kc + 1)].T
        wb_cat[kc, :, 256:] = aw_w[:, P * kc:P * (kc + 1)].T
    pj_t = np.empty((2, 2, P, P), np.float32)
    for kc in range(2):
        for mc in range(2):
            pj_t[kc, mc] = proj_w[P * mc:P * (mc + 1), P * kc:P * (kc + 1)].T
    pb_t = proj_b.reshape(2, P)
    ident = np.eye(P, dtype=np.float32)
    blob = np.concatenate(
        [s_mats[s] for s in range(s_mats.shape[0])], axis=1)
    blob2 = np.concatenate(
        [pj_t[kc, mc] for kc in range(2) for mc in range(2)]
        + [ident], axis=1)
    shared = dict(
        consts=np.ascontiguousarray(blob.astype(bf16)),
        consts2=np.ascontiguousarray(blob2.astype(bf16)),
        wb_cat=np.ascontiguousarray(
            np.concatenate([wb_cat[0], wb_cat[1]], axis=1).astype(bf16)))
    if has_pbias:
        shared["projb_t"] = np.ascontiguousarray(pb_t)
    if has_bias:
        bb_cat = np.concatenate([v_b, aw_b]).reshape(1, NCH)
        shared["bb_cat"] = np.ascontiguousarray(bb_cat.astype(bf16))

    xr = x.reshape(B, H, W, DIM)
    in_maps = []
    for d in range(N_CORES):
        b, half = d // 2, d % 2
        r0 = ROWS_OUT * half
        x_dev = np.zeros((ROWS_V, W, DIM), np.float32)
        lo, hi = max(0, r0 - HALO), min(H, r0 + ROWS_OUT + HALO)
        x_dev[lo - (r0 - HALO):hi - (r0 - HALO)] = xr[b, lo:hi]
        m = dict(shared)
        m["xt_dev"] = np.ascontiguousarray(
            x_dev.reshape(TOK_V, DIM).T.astype(bf16))
        if has_bias:
            ones = np.zeros((ROWS_V, W), np.float32)
            ones[lo - (r0 - HALO):hi - (r0 - HALO)] = 1.0
            m["ones_dev"] = ones.reshape(1, TOK_V).astype(bf16)
        in_maps.append(m)

    from concourse import bass_utils
    res = bass_utils.run_bass_kernel_spmd(
        nc, in_maps, core_ids=list(range(N_CORES)),
        trace=os.environ.get("KERNEL_TRACE", "0") == "1")
    kernel.last_results = res

    y = np.empty((B, N_TOK, DIM), np.float32)
    for d in range(N_CORES):
        b, half = d // 2, d % 2
        yd = np.concatenate([res.results[d]["y0"], res.results[d]["y1"]], 0)
        y[b, ROWS_OUT * W * half:ROWS_OUT * W * (half + 1), :] = yd.T
    return y
